# revision 7
# baseline (speedup 1.0000x reference)
"""Trainium2 Bass kernel for an attention block with a non-standard
(query-axis) softmax and causal mask.

Math per batch element b (T=2048 tokens, C=K=V=512):
    q = x @ Wq.T + bq ; k = x @ Wk.T + bk ; v = x @ Wv.T + bv
    logits[j, i] = q[j] . k[i]                     (j=query, i=key)
    masked = -inf where i > j
    probs = softmax(masked / sqrt(512), axis=j)    <-- softmax over QUERY axis
    read[j] = sum_i probs[j, i] * v[i]
    out = concat(x, read)                          [T, 1024]

Distribution: pure data-parallel, batch b -> core b (8 batches, 8 cores),
weights replicated, no collectives.  The passthrough half of the output is
concatenated on the host; the device computes and returns only `read`.

Layout: compute L^T[i, j] (key index i on partitions, query index j on the
free dim); the axis=1 softmax reduces along the free dim, which ACT fuses
into the exp via accum_out.  Only j-chunks at or right of the diagonal are
computed.

fp8 DoubleRow everywhere: all five matmul stages run in fp8_e4m3 with
perf_mode=DoubleRow (256-deep contraction per instruction, 2x the bf16 PE
rate; a [128,2,M]x[128,2,N] instruction measures 216ns at N=512).
Operands are pair-interleaved [128, 2, N]: partition p of pair g holds
contraction rows 256g+p and 256g+128+p.

fp8 conditioning: weights/x are cast raw (values straddle the e4m3
denormal cutoff but abs quantization error stays ~2^-10, the same
3-4%-of-sigma noise as the normal range).  E = exp(logits/sqrt(512)) lands
in [0, 3] so the exp writes fp8 E-hat tiles directly.  The softmax
normalizer rides V': V' = V * (rho * 32) with rho = min(1/S, 2.5); the
global *32 shift keeps typical products above the fp8 denormal floor and
the read-out copy multiplies by 1/32.  Biases are exact: Q/K bias is added
during the PSUM->fp8 convert (ACT Identity with per-partition bias, split
with DVE tensor_scalar to balance engines), V bias by DVE from a
pre-broadcast bf16 tile.

The causal mask is applied by the PE itself: a ones-triangle lhsT times a
one-hot -1e30 rhs constant seeds the logits PSUM as an extra bf16 matmul
opening the diagonal chunk's accumulation group (no DVE op).

Scheduling notes (from perfetto trace analysis):
- DMA *issue* is serial on a sequencer (~0.6us per dma_start), so loads
  are batched into a handful of whole-tensor DMAs split across the sync
  HWDGE queue (x^T) and the gpsimd SWDGE queue (weights + consts).
- Each stationary weight is reused across all four 512-col moving chunks
  (ldweights amortized 4x) in both the projection and logits stages --
  back-to-back weight switches outrun the PE's weight prefetch and cost
  ~146ns/instr.
- V-projection chains are emitted inside phase 2 (V[it] right after
  logits[it]) so the PE fills the stalls where ACT's exp pipeline lags.
- Output DMAs ride the sync HWDGE queue, which is idle after the loads.
- Full-width warm-up matmuls on a memset tile run during the initial load
  so the PE's HAM clock gate is at full rate when real work arrives.
"""

import math

import numpy as np
import ml_dtypes

P = 128
B, T, C = 8, 2048, 512
NT = T // P     # 16 row tiles
NJ = T // 512   # 4 query chunks of 512
NH = NT // 2    # 8 contraction pairs for the read matmul
NCORES = 8
NEG = -1e30

_BUILT = None


def _build_nc():
    import concourse.mybir as mybir
    import concourse.tile as tile
    from concourse import bacc

    f32 = mybir.dt.float32
    bf16 = mybir.dt.bfloat16
    fp8 = mybir.dt.float8e4
    AF = mybir.ActivationFunctionType
    DR = mybir.MatmulPerfMode.DoubleRow
    ALU = mybir.AluOpType
    S_EXP = 1.0 / math.sqrt(C)

    nc = bacc.Bacc("TRN2", target_bir_lowering=False, debug=False,
                   num_devices=NCORES)

    # Pair-interleaved fp8 operands: [p, g, i, n] = M[256g + 128i + p, n].
    xt_d = nc.dram_tensor("xt8", [P, 2, 2, T], fp8, kind="ExternalInput")
    # All three weights in one tensor: [p, (wq|wk|wv), g, i, kout].
    w_d = nc.dram_tensor("w8", [P, 3, 2, 2, C], fp8, kind="ExternalInput")
    bqk_d = nc.dram_tensor("bqk", [P, 8], f32, kind="ExternalInput")
    # bf16 consts packed: [p, 0:512]=bv broadcast, [512:640]=tri,
    # [640:2688]=mask rhs.  tri[r, p] = [p >= r]; mrhs holds one-hot -1e30
    # columns per diagonal sub-position m (see _make_in_maps).
    cb_d = nc.dram_tensor("cb16", [P, 512 + P + 4 * 512], bf16,
                          kind="ExternalInput")
    out_d = nc.dram_tensor("out", [T, C], f32, kind="ExternalOutput")

    with tile.TileContext(nc) as tc:
        with (
            tc.tile_pool(name="const", bufs=1) as cpool,
            tc.tile_pool(name="w", bufs=1) as wpool,
            tc.tile_pool(name="xt", bufs=1) as xtpool,
            tc.tile_pool(name="qt", bufs=1) as qtpool,
            tc.tile_pool(name="kt", bufs=1) as ktpool,
            tc.tile_pool(name="v", bufs=1) as vpool,
            tc.tile_pool(name="vp", bufs=1) as vppool,
            tc.tile_pool(name="et", bufs=1) as etpool,
            tc.tile_pool(name="small", bufs=8) as spool,
            tc.tile_pool(name="ostage", bufs=4) as ospool,
        ):
            # --- loads: two issue queues in parallel ---
            # sync HWDGE: x^T (the critical path for the first matmuls)
            xt_t = [xtpool.tile([P, 2, T], fp8, name=f"xt{g}", tag=f"xt{g}")
                    for g in range(2)]
            for g in range(2):
                nc.sync.dma_start(xt_t[g][:], xt_d[:, g, :, :])
            # gpsimd SWDGE: weights + consts
            w_t = wpool.tile([P, 3, 2, 2, C], fp8, name="w_t")
            nc.gpsimd.dma_start(w_t[:], w_d[:])
            bqk_t = cpool.tile([P, 8], f32, name="bqk_t")
            nc.gpsimd.dma_start(bqk_t[:], bqk_d[:])
            cb_t = cpool.tile([P, 512 + P + 4 * 512], bf16, name="cb_t")
            nc.gpsimd.dma_start(cb_t[:], cb_d[:])
            bvf_t = cb_t[:, 0:512]
            tri_t = cb_t[:, 512:512 + P]
            mrhs_t = cb_t[:, 512 + P:512 + P + 4 * 512]

            def wsl(which, g, kcols):  # weight slice [128, 2, kcols]
                return w_t[:, which, g, :, kcols]

            with tc.tile_pool(name="psqk", bufs=6, space="PSUM") as psqk:
                # PE warm-up: junk matmuls with NO DMA dependency (source
                # is memset on-chip) so they start right after the NEFF
                # prologue; full-width so the HAM activity monitor sees
                # them.
                warm_src = cpool.tile([P, C + P], bf16, name="warm_src")
                nc.vector.memset(warm_src[:], 0.0)
                ps_warm = psqk.tile([P, 512], f32, name="ps_warm", tag="psqk")
                for _ in range(11):
                    nc.tensor.matmul(ps_warm[:], warm_src[:, C:C + P],
                                     warm_src[:, 0:C], start=True, stop=True)

                # --- Phase 1: Q^T, K^T pair-interleaved fp8 [k, t] ---
                # Q^T[k, t] = sum_c WqT[c, k] * XT[c, t].  Each stationary
                # weight slice sweeps all four 512-col j-chunks before the
                # PE switches weights.  Bias + fp8 convert: ACT (Identity,
                # per-partition bias) for Q kb 0-2, DVE for the rest.
                qt_t = [qtpool.tile([P, 2, T], fp8, name=f"qt{g}",
                                    tag=f"qt{g}") for g in range(2)]
                kt_t = [ktpool.tile([P, 2, T], fp8, name=f"kt{g}",
                                    tag=f"kt{g}") for g in range(2)]
                for kb in range(4):
                    ksl = slice(kb * P, (kb + 1) * P)
                    for dst, which, bcol in ((qt_t, 0, kb), (kt_t, 1, 4 + kb)):
                        pss = [psqk.tile([P, 512], f32, name=f"ps{jc}",
                                         tag="psqk") for jc in range(NJ)]
                        for g in range(2):
                            for jc in range(NJ):
                                js = slice(jc * 512, (jc + 1) * 512)
                                nc.tensor.matmul(pss[jc][:],
                                                 wsl(which, g, ksl),
                                                 xt_t[g][:, :, js],
                                                 start=(g == 0),
                                                 stop=(g == 1),
                                                 perf_mode=DR)
                        for jc in range(NJ):
                            js = slice(jc * 512, (jc + 1) * 512)
                            if which == 0 and kb < 3:
                                nc.scalar.activation(
                                    dst[kb // 2][:, kb % 2, js], pss[jc][:],
                                    AF.Identity,
                                    bias=bqk_t[:, bcol:bcol + 1])
                            else:
                                nc.vector.tensor_scalar_add(
                                    dst[kb // 2][:, kb % 2, js], pss[jc][:],
                                    bqk_t[:, bcol:bcol + 1])

            # --- Phase 2: masked logits + exp(fp8) + row sums + V, V' ---
            # E-hat pair tiles: [p, i, j] = E[256h + 128i + p, j].
            et_t = [etpool.tile([P, 2, T], fp8, name=f"et{h}", tag=f"et{h}")
                    for h in range(NH)]
            vp_t = [vppool.tile([P, 2, 512], fp8, name=f"vp{h}", tag=f"vp{h}")
                    for h in range(NH)]
            # The pair (2h, 2h+1) is read over the jt=2h diagonal block where
            # sub-row 1 (tile 2h+1) is below its own trim: zero it once.
            for h in range(NH):
                nc.gpsimd.memset(et_t[h][:, 1, 256 * h:256 * h + P], 0.0)

            with (
                tc.tile_pool(name="psl", bufs=6, space="PSUM") as psl,
                tc.tile_pool(name="pso", bufs=2, space="PSUM") as pso,
            ):
                v_t = []
                for it in range(NT):
                    jc0 = it // 4
                    m = it % 4
                    isl = slice(it * P, (it + 1) * P)
                    h, sub = it // 2, it % 2
                    nck = NJ - jc0  # chunks for this row tile
                    pss = [psl.tile([P, 512], f32, name=f"psl{jc}",
                                    tag="psl") for jc in range(nck)]
                    off0 = 128 * m
                    # seed the diagonal chunk's PSUM with the causal mask
                    # (opens its group; the QK matmuls accumulate on top)
                    nc.tensor.matmul(
                        pss[0][:, 0:512 - off0], tri_t,
                        mrhs_t[:, m * 512 + off0:(m + 1) * 512],
                        start=True, stop=False, skip_group_check=True)
                    for g in range(2):
                        for cx in range(nck):
                            jc = jc0 + cx
                            off = off0 if cx == 0 else 0
                            w = 512 - off
                            js = slice(jc * 512 + off, (jc + 1) * 512)
                            nc.tensor.matmul(pss[cx][:, 0:w],
                                             kt_t[g][:, :, isl],
                                             qt_t[g][:, :, js],
                                             start=(g == 0 and cx != 0),
                                             stop=(g == 1),
                                             perf_mode=DR,
                                             skip_group_check=(cx == 0))
                    parts = []
                    for cx in range(nck):
                        jc = jc0 + cx
                        off = off0 if cx == 0 else 0
                        w = 512 - off
                        js = slice(jc * 512 + off, (jc + 1) * 512)
                        part = spool.tile([P, 1], f32, name="part",
                                          tag="part")
                        nc.scalar.activation(et_t[h][:, sub, js],
                                             pss[cx][:, 0:w],
                                             AF.Exp, scale=S_EXP,
                                             accum_out=part[:])
                        parts.append(part)
                    # V[it] emitted here: the PE runs it where ACT lags.
                    psV = pso.tile([P, 512], f32, name="psV", tag="pso")
                    for g in range(2):
                        nc.tensor.matmul(psV[:],
                                         xt_t[g][:, :, it * P:(it + 1) * P],
                                         wsl(2, g, slice(0, C)),
                                         start=(g == 0), stop=(g == 1),
                                         perf_mode=DR)
                    vt = vpool.tile([P, 512], bf16, name=f"v{it}",
                                    tag=f"v{it}")
                    nc.vector.tensor_add(vt[:], psV[:], bvf_t)
                    v_t.append(vt)
                    if len(parts) == 1:
                        s = parts[0]
                    else:
                        s = spool.tile([P, 1], f32, name="s", tag="s")
                        nc.vector.tensor_add(s[:], parts[0][:], parts[1][:])
                        for p_ in parts[2:]:
                            nc.vector.tensor_add(s[:], s[:], p_[:])
                    r = spool.tile([P, 1], f32, name="r", tag="r")
                    nc.vector.reciprocal(r[:], s[:])
                    # rho32 = min(1/S, 2.5) * 32, fused
                    r32 = spool.tile([P, 1], f32, name="r32", tag="r32")
                    nc.vector.tensor_scalar(r32[:], r[:], 32.0, 80.0,
                                            op0=ALU.mult, op1=ALU.min)
                    nc.vector.tensor_scalar_mul(vp_t[h][:, sub, :], vt[:],
                                                r32[:])

                # --- Phase 3: read[jt] = sum_h Ehat[h][:,:,jsl].T @ V'[h]
                # The last two rows split their accumulation into two
                # chains so the post-phase-2 critical path is short.
                for jt in range(NT):
                    jsl = slice(jt * P, (jt + 1) * P)
                    nh = (jt + 2) // 2  # pairs covering it <= jt
                    ost = ospool.tile([P, 512], f32, name="ost", tag="ost")
                    if jt >= NT - 2:
                        ha = nh // 2
                        psa = psl.tile([P, 512], f32, name="psa", tag="psl")
                        for h in range(ha):
                            nc.tensor.matmul(psa[:], et_t[h][:, :, jsl],
                                             vp_t[h][:, :, :],
                                             start=(h == 0),
                                             stop=(h == ha - 1),
                                             perf_mode=DR)
                        # stage the early half in SBUF, pre-scaled by 1/32
                        sba = ospool.tile([P, 512], f32, name="sba",
                                          tag="sba")
                        nc.vector.tensor_scalar_mul(sba[:], psa[:],
                                                    1.0 / 32.0)
                        psb = pso.tile([P, 512], f32, name="psb", tag="pso")
                        for h in range(ha, nh):
                            nc.tensor.matmul(psb[:], et_t[h][:, :, jsl],
                                             vp_t[h][:, :, :],
                                             start=(h == ha),
                                             stop=(h == nh - 1),
                                             perf_mode=DR)
                        nc.vector.scalar_tensor_tensor(
                            ost[:], psb[:], 1.0 / 32.0, sba[:],
                            op0=ALU.mult, op1=ALU.add)
                    else:
                        ps = pso.tile([P, 512], f32, name="pso", tag="pso")
                        for h in range(nh):
                            nc.tensor.matmul(ps[:], et_t[h][:, :, jsl],
                                             vp_t[h][:, :, :],
                                             start=(h == 0),
                                             stop=(h == nh - 1),
                                             perf_mode=DR)
                        nc.vector.tensor_scalar_mul(ost[:], ps[:],
                                                    1.0 / 32.0)
                    nc.sync.dma_start(out_d[jsl, :], ost[:])

    nc.compile()
    return nc


def _get_built():
    global _BUILT
    if _BUILT is None:
        _BUILT = _build_nc()
    return _BUILT


def _pair_interleave(mat):
    """[512, N] -> [128, 2, 2, N] with [p, g, i, :] = mat[256g + 128i + p]."""
    n = mat.shape[1]
    return np.ascontiguousarray(
        mat.reshape(2, 2, P, n).transpose(2, 0, 1, 3))


def _make_in_maps(input, Wq, bq, Wk, bk, Wv, bv):
    bf = ml_dtypes.bfloat16
    f8 = ml_dtypes.float8_e4m3

    input = np.asarray(input, np.float32)
    Wq = np.asarray(Wq, np.float32)
    bq = np.asarray(bq, np.float32)
    Wk = np.asarray(Wk, np.float32)
    bk = np.asarray(bk, np.float32)
    Wv = np.asarray(Wv, np.float32)
    bv = np.asarray(bv, np.float32)

    w8 = np.stack([_pair_interleave(np.ascontiguousarray(W.T))
                   for W in (Wq, Wk, Wv)], axis=1).astype(f8)

    bqk = np.empty((P, 8), np.float32)
    for kb in range(4):
        bqk[:, kb] = bq[kb * P:(kb + 1) * P]
        bqk[:, 4 + kb] = bk[kb * P:(kb + 1) * P]

    # bf16 consts: bv broadcast | tri | mask rhs
    cb = np.zeros((P, 512 + P + 4 * 512), np.float32)
    cb[:, 0:512] = bv[None, :]
    rr = np.arange(P)[:, None]
    pp = np.arange(P)[None, :]
    cb[:, 512:512 + P] = (pp >= rr)
    # Mask-as-matmul: out[p, x] = sum_r tri[r, p] * mrhs[r, m*512 + x]
    #               = NEG * [x < p + 128*m].
    for m in range(4):
        for x in range(512):
            t = x - 128 * m + 1
            if x < 128 * m:
                cb[0, 512 + P + m * 512 + x] = NEG
            elif t <= P - 1:
                cb[t, 512 + P + m * 512 + x] = NEG
    cb = cb.astype(bf)

    in_maps = []
    for b in range(B):
        xb = np.ascontiguousarray(input[b])
        in_maps.append({
            "xt8": _pair_interleave(np.ascontiguousarray(xb.T)).astype(f8),
            "w8": w8, "bqk": bqk, "cb16": cb,
        })
    return in_maps


def kernel(input, Wq, bq, Wk, bk, Wv, bv, _trace=False):
    from concourse.bass_utils import run_bass_kernel_spmd

    nc = _get_built()
    input = np.asarray(input, np.float32)
    in_maps = _make_in_maps(input, Wq, bq, Wk, bk, Wv, bv)
    res = run_bass_kernel_spmd(nc, in_maps, core_ids=list(range(NCORES)),
                               trace=_trace)
    read = np.stack([r["out"] for r in res.results], axis=0)
    out = np.concatenate((input, read), axis=2)
    if _trace:
        kernel.last_result = res
    return out


# revision 10
# speedup vs baseline: 1.0317x; 1.0317x over previous
"""Trainium2 Bass kernel for an attention block with a non-standard
(query-axis) softmax and causal mask.

Math per batch element b (T=2048 tokens, C=K=V=512):
    q = x @ Wq.T + bq ; k = x @ Wk.T + bk ; v = x @ Wv.T + bv
    logits[j, i] = q[j] . k[i]                     (j=query, i=key)
    masked = -inf where i > j
    probs = softmax(masked / sqrt(512), axis=j)    <-- softmax over QUERY axis
    read[j] = sum_i probs[j, i] * v[i]
    out = concat(x, read)                          [T, 1024]

Distribution: pure data-parallel, batch b -> core b (8 batches, 8 cores),
weights replicated, no collectives.  The passthrough half of the output is
concatenated on the host; the device computes and returns only `read`.

Layout: compute L^T[i, j] (key index i on partitions, query index j on the
free dim); the axis=1 softmax reduces along the free dim, which ACT fuses
into the exp via accum_out.  Only j-chunks at or right of the diagonal are
computed.

fp8 DoubleRow everywhere: all five matmul stages run in fp8_e4m3 with
perf_mode=DoubleRow (256-deep contraction per instruction, 2x the bf16 PE
rate; a [128,2,M]x[128,2,N] instruction measures 216ns at N=512).
Operands are pair-interleaved [128, 2, N]: partition p of pair g holds
contraction rows 256g+p and 256g+128+p.

fp8 conditioning: weights/x are cast raw (values straddle the e4m3
denormal cutoff but abs quantization error stays ~2^-10, the same
3-4%-of-sigma noise as the normal range).  E = exp(logits/sqrt(512)) lands
in [0, 3] so the exp writes fp8 E-hat tiles directly.  The softmax
normalizer rides V': V' = V * (rho * 32) with rho = min(1/S, 2.5); the
global *32 shift keeps typical products above the fp8 denormal floor and
the read-out copy multiplies by 1/32.  Biases are exact: Q/K bias is added
during the PSUM->fp8 convert (ACT Identity with per-partition bias, split
with DVE tensor_scalar to balance engines), V bias by DVE from a
pre-broadcast bf16 tile.

The causal mask is applied by the PE itself: a ones-triangle lhsT times a
one-hot -1e30 rhs constant seeds the logits PSUM as an extra bf16 matmul
opening the diagonal chunk's accumulation group (no DVE op).

Scheduling notes (from perfetto trace analysis):
- DMA *issue* is serial on a sequencer (~0.6us per dma_start), so loads
  are batched into a handful of whole-tensor DMAs split across the sync
  HWDGE queue (x^T) and the gpsimd SWDGE queue (weights + consts).
- Each stationary weight is reused across all four 512-col moving chunks
  (ldweights amortized 4x) in both the projection and logits stages --
  back-to-back weight switches outrun the PE's weight prefetch and cost
  ~146ns/instr.
- V-projection chains are emitted inside phase 2 (V[it] right after
  logits[it]) so the PE fills the stalls where ACT's exp pipeline lags.
- Output DMAs ride the sync HWDGE queue, which is idle after the loads.
- Full-width warm-up matmuls on a memset tile run during the initial load
  so the PE's HAM clock gate is at full rate when real work arrives.
"""

import math

import numpy as np
import ml_dtypes

P = 128
B, T, C = 8, 2048, 512
NT = T // P     # 16 row tiles
NJ = T // 512   # 4 query chunks of 512
NH = NT // 2    # 8 contraction pairs for the read matmul
NCORES = 8
NEG = -1e30

_BUILT = None


def _build_nc():
    import concourse.mybir as mybir
    import concourse.tile as tile
    from concourse import bacc

    f32 = mybir.dt.float32
    bf16 = mybir.dt.bfloat16
    fp8 = mybir.dt.float8e4
    AF = mybir.ActivationFunctionType
    DR = mybir.MatmulPerfMode.DoubleRow
    ALU = mybir.AluOpType
    S_EXP = 1.0 / math.sqrt(C)

    nc = bacc.Bacc("TRN2", target_bir_lowering=False, debug=False,
                   num_devices=NCORES)

    # Pair-interleaved fp8 operands: [p, g, i, n] = M[256g + 128i + p, n].
    xt_d = nc.dram_tensor("xt8", [P, 2, 2, T], fp8, kind="ExternalInput")
    # All three weights in one tensor: [p, (wq|wk|wv), g, i, kout].
    w_d = nc.dram_tensor("w8", [P, 3, 2, 2, C], fp8, kind="ExternalInput")
    bqk_d = nc.dram_tensor("bqk", [P, 8], f32, kind="ExternalInput")
    # bf16 consts packed: [p, 0:512]=bv broadcast, [512:640]=tri,
    # [640:2688]=mask rhs.  tri[r, p] = [p >= r]; mrhs holds one-hot -1e30
    # columns per diagonal sub-position m (see _make_in_maps).
    cb_d = nc.dram_tensor("cb16", [P, 512 + P + 4 * 512], bf16,
                          kind="ExternalInput")
    out_d = nc.dram_tensor("out", [T, C], f32, kind="ExternalOutput")

    with tile.TileContext(nc) as tc:
        with (
            tc.tile_pool(name="const", bufs=1) as cpool,
            tc.tile_pool(name="w", bufs=1) as wpool,
            tc.tile_pool(name="xt", bufs=1) as xtpool,
            tc.tile_pool(name="qt", bufs=1) as qtpool,
            tc.tile_pool(name="kt", bufs=1) as ktpool,
            tc.tile_pool(name="v", bufs=1) as vpool,
            tc.tile_pool(name="vp", bufs=1) as vppool,
            tc.tile_pool(name="et", bufs=1) as etpool,
            tc.tile_pool(name="small", bufs=8) as spool,
            tc.tile_pool(name="ostage", bufs=4) as ospool,
        ):
            # --- loads: three HWDGE issue queues in parallel ---
            # (sw-DGE is NOT used for loads: the Pool engine's first DMA
            # carries a ~13us ring-init latency)
            xt_t = [xtpool.tile([P, 2, T], fp8, name=f"xt{g}", tag=f"xt{g}")
                    for g in range(2)]
            nc.sync.dma_start(xt_t[0][:], xt_d[:, 0, :, :])
            nc.sync.dma_start(xt_t[1][:], xt_d[:, 1, :, :])
            w_t = wpool.tile([P, 3, 2, 2, C], fp8, name="w_t")
            nc.scalar.dma_start(w_t[:], w_d[:])
            bqk_t = cpool.tile([P, 8], f32, name="bqk_t")
            nc.scalar.dma_start(bqk_t[:], bqk_d[:])
            cb_t = cpool.tile([P, 512 + P + 4 * 512], bf16, name="cb_t")
            nc.scalar.dma_start(cb_t[:], cb_d[:])
            bvf_t = cb_t[:, 0:512]
            tri_t = cb_t[:, 512:512 + P]
            mrhs_t = cb_t[:, 512 + P:512 + P + 4 * 512]

            def wsl(which, g, kcols):  # weight slice [128, 2, kcols]
                return w_t[:, which, g, :, kcols]

            with tc.tile_pool(name="psqk", bufs=6, space="PSUM") as psqk:
                # PE warm-up: junk matmuls with NO DMA dependency (source
                # is memset on-chip) so they start right after the NEFF
                # prologue; full-width so the HAM activity monitor sees
                # them.
                warm_src = cpool.tile([P, C + P], bf16, name="warm_src")
                nc.vector.memset(warm_src[:], 0.0)
                ps_warm = psqk.tile([P, 512], f32, name="ps_warm", tag="psqk")
                for _ in range(11):
                    nc.tensor.matmul(ps_warm[:], warm_src[:, C:C + P],
                                     warm_src[:, 0:C], start=True, stop=True)

                # --- Phase 1: Q^T, K^T pair-interleaved fp8 [k, t] ---
                # Q^T[k, t] = sum_c WqT[c, k] * XT[c, t].  Each stationary
                # weight slice sweeps all four 512-col j-chunks before the
                # PE switches weights.  Bias + fp8 convert: ACT (Identity,
                # per-partition bias) for Q kb 0-2, DVE for the rest.
                qt_t = [qtpool.tile([P, 2, T], fp8, name=f"qt{g}",
                                    tag=f"qt{g}") for g in range(2)]
                kt_t = [ktpool.tile([P, 2, T], fp8, name=f"kt{g}",
                                    tag=f"kt{g}") for g in range(2)]
                for kb in range(4):
                    ksl = slice(kb * P, (kb + 1) * P)
                    for dst, which, bcol in ((qt_t, 0, kb), (kt_t, 1, 4 + kb)):
                        pss = [psqk.tile([P, 512], f32, name=f"ps{jc}",
                                         tag="psqk") for jc in range(NJ)]
                        for g in range(2):
                            for jc in range(NJ):
                                js = slice(jc * 512, (jc + 1) * 512)
                                nc.tensor.matmul(pss[jc][:],
                                                 wsl(which, g, ksl),
                                                 xt_t[g][:, :, js],
                                                 start=(g == 0),
                                                 stop=(g == 1),
                                                 perf_mode=DR)
                        for jc in range(NJ):
                            js = slice(jc * 512, (jc + 1) * 512)
                            if which == 0 and kb < 3:
                                nc.scalar.activation(
                                    dst[kb // 2][:, kb % 2, js], pss[jc][:],
                                    AF.Identity,
                                    bias=bqk_t[:, bcol:bcol + 1])
                            else:
                                nc.vector.tensor_scalar_add(
                                    dst[kb // 2][:, kb % 2, js], pss[jc][:],
                                    bqk_t[:, bcol:bcol + 1])

            # --- Phase 2: masked logits + exp(fp8) + row sums + V, V' ---
            # E-hat pair tiles: [p, i, j] = E[256h + 128i + p, j].
            et_t = [etpool.tile([P, 2, T], fp8, name=f"et{h}", tag=f"et{h}")
                    for h in range(NH)]
            vp_t = [vppool.tile([P, 2, 512], fp8, name=f"vp{h}", tag=f"vp{h}")
                    for h in range(NH)]
            # The pair (2h, 2h+1) is read over the jt=2h diagonal block where
            # sub-row 1 (tile 2h+1) is below its own trim: zero it once.
            for h in range(NH):
                nc.gpsimd.memset(et_t[h][:, 1, 256 * h:256 * h + P], 0.0)

            with (
                tc.tile_pool(name="psl", bufs=6, space="PSUM") as psl,
                tc.tile_pool(name="pso", bufs=2, space="PSUM") as pso,
            ):
                v_t = []
                for it in range(NT):
                    jc0 = it // 4
                    m = it % 4
                    isl = slice(it * P, (it + 1) * P)
                    h, sub = it // 2, it % 2
                    nck = NJ - jc0  # chunks for this row tile
                    pss = [psl.tile([P, 512], f32, name=f"psl{jc}",
                                    tag="psl") for jc in range(nck)]
                    off0 = 128 * m
                    # seed the diagonal chunk's PSUM with the causal mask
                    # (opens its group; the QK matmuls accumulate on top)
                    nc.tensor.matmul(
                        pss[0][:, 0:512 - off0], tri_t,
                        mrhs_t[:, m * 512 + off0:(m + 1) * 512],
                        start=True, stop=False, skip_group_check=True)
                    for g in range(2):
                        for cx in range(nck):
                            jc = jc0 + cx
                            off = off0 if cx == 0 else 0
                            w = 512 - off
                            js = slice(jc * 512 + off, (jc + 1) * 512)
                            nc.tensor.matmul(pss[cx][:, 0:w],
                                             kt_t[g][:, :, isl],
                                             qt_t[g][:, :, js],
                                             start=(g == 0 and cx != 0),
                                             stop=(g == 1),
                                             perf_mode=DR,
                                             skip_group_check=(cx == 0))
                    parts = []
                    for cx in range(nck):
                        jc = jc0 + cx
                        off = off0 if cx == 0 else 0
                        w = 512 - off
                        js = slice(jc * 512 + off, (jc + 1) * 512)
                        part = spool.tile([P, 1], f32, name="part",
                                          tag="part")
                        nc.scalar.activation(et_t[h][:, sub, js],
                                             pss[cx][:, 0:w],
                                             AF.Exp, scale=S_EXP,
                                             accum_out=part[:])
                        parts.append(part)
                    # V[it] emitted here: the PE runs it where ACT lags.
                    psV = pso.tile([P, 512], f32, name="psV", tag="pso")
                    for g in range(2):
                        nc.tensor.matmul(psV[:],
                                         xt_t[g][:, :, it * P:(it + 1) * P],
                                         wsl(2, g, slice(0, C)),
                                         start=(g == 0), stop=(g == 1),
                                         perf_mode=DR)
                    vt = vpool.tile([P, 512], bf16, name=f"v{it}",
                                    tag=f"v{it}")
                    nc.vector.tensor_add(vt[:], psV[:], bvf_t)
                    v_t.append(vt)
                    if len(parts) == 1:
                        s = parts[0]
                    else:
                        s = spool.tile([P, 1], f32, name="s", tag="s")
                        nc.vector.tensor_add(s[:], parts[0][:], parts[1][:])
                        for p_ in parts[2:]:
                            nc.vector.tensor_add(s[:], s[:], p_[:])
                    r = spool.tile([P, 1], f32, name="r", tag="r")
                    nc.vector.reciprocal(r[:], s[:])
                    # rho32 = min(1/S, 2.5) * 32, fused
                    r32 = spool.tile([P, 1], f32, name="r32", tag="r32")
                    nc.vector.tensor_scalar(r32[:], r[:], 32.0, 80.0,
                                            op0=ALU.mult, op1=ALU.min)
                    nc.vector.tensor_scalar_mul(vp_t[h][:, sub, :], vt[:],
                                                r32[:])

                # --- Phase 3: read[jt] = sum_h Ehat[h][:,:,jsl].T @ V'[h]
                # The last two rows split their accumulation into two
                # chains so the post-phase-2 critical path is short.
                for jt in range(NT):
                    jsl = slice(jt * P, (jt + 1) * P)
                    nh = (jt + 2) // 2  # pairs covering it <= jt
                    ost = ospool.tile([P, 512], f32, name="ost", tag="ost")
                    if jt >= NT - 2:
                        ha = nh // 2
                        psa = psl.tile([P, 512], f32, name="psa", tag="psl")
                        for h in range(ha):
                            nc.tensor.matmul(psa[:], et_t[h][:, :, jsl],
                                             vp_t[h][:, :, :],
                                             start=(h == 0),
                                             stop=(h == ha - 1),
                                             perf_mode=DR)
                        # stage the early half in SBUF, pre-scaled by 1/32
                        sba = ospool.tile([P, 512], f32, name="sba",
                                          tag="sba")
                        nc.vector.tensor_scalar_mul(sba[:], psa[:],
                                                    1.0 / 32.0)
                        psb = pso.tile([P, 512], f32, name="psb", tag="pso")
                        for h in range(ha, nh):
                            nc.tensor.matmul(psb[:], et_t[h][:, :, jsl],
                                             vp_t[h][:, :, :],
                                             start=(h == ha),
                                             stop=(h == nh - 1),
                                             perf_mode=DR)
                        nc.vector.scalar_tensor_tensor(
                            ost[:], psb[:], 1.0 / 32.0, sba[:],
                            op0=ALU.mult, op1=ALU.add)
                    else:
                        ps = pso.tile([P, 512], f32, name="pso", tag="pso")
                        for h in range(nh):
                            nc.tensor.matmul(ps[:], et_t[h][:, :, jsl],
                                             vp_t[h][:, :, :],
                                             start=(h == 0),
                                             stop=(h == nh - 1),
                                             perf_mode=DR)
                        nc.vector.tensor_scalar_mul(ost[:], ps[:],
                                                    1.0 / 32.0)
                    nc.gpsimd.dma_start(out_d[jsl, :], ost[:])

    nc.compile()
    return nc


def _get_built():
    global _BUILT
    if _BUILT is None:
        _BUILT = _build_nc()
    return _BUILT


def _pair_interleave(mat):
    """[512, N] -> [128, 2, 2, N] with [p, g, i, :] = mat[256g + 128i + p]."""
    n = mat.shape[1]
    return np.ascontiguousarray(
        mat.reshape(2, 2, P, n).transpose(2, 0, 1, 3))


def _make_in_maps(input, Wq, bq, Wk, bk, Wv, bv):
    bf = ml_dtypes.bfloat16
    f8 = ml_dtypes.float8_e4m3

    input = np.asarray(input, np.float32)
    Wq = np.asarray(Wq, np.float32)
    bq = np.asarray(bq, np.float32)
    Wk = np.asarray(Wk, np.float32)
    bk = np.asarray(bk, np.float32)
    Wv = np.asarray(Wv, np.float32)
    bv = np.asarray(bv, np.float32)

    w8 = np.stack([_pair_interleave(np.ascontiguousarray(W.T))
                   for W in (Wq, Wk, Wv)], axis=1).astype(f8)

    bqk = np.empty((P, 8), np.float32)
    for kb in range(4):
        bqk[:, kb] = bq[kb * P:(kb + 1) * P]
        bqk[:, 4 + kb] = bk[kb * P:(kb + 1) * P]

    # bf16 consts: bv broadcast | tri | mask rhs
    cb = np.zeros((P, 512 + P + 4 * 512), np.float32)
    cb[:, 0:512] = bv[None, :]
    rr = np.arange(P)[:, None]
    pp = np.arange(P)[None, :]
    cb[:, 512:512 + P] = (pp >= rr)
    # Mask-as-matmul: out[p, x] = sum_r tri[r, p] * mrhs[r, m*512 + x]
    #               = NEG * [x < p + 128*m].
    for m in range(4):
        for x in range(512):
            t = x - 128 * m + 1
            if x < 128 * m:
                cb[0, 512 + P + m * 512 + x] = NEG
            elif t <= P - 1:
                cb[t, 512 + P + m * 512 + x] = NEG
    cb = cb.astype(bf)

    in_maps = []
    for b in range(B):
        xb = np.ascontiguousarray(input[b])
        in_maps.append({
            "xt8": _pair_interleave(np.ascontiguousarray(xb.T)).astype(f8),
            "w8": w8, "bqk": bqk, "cb16": cb,
        })
    return in_maps


def kernel(input, Wq, bq, Wk, bk, Wv, bv, _trace=False):
    from concourse.bass_utils import run_bass_kernel_spmd

    nc = _get_built()
    input = np.asarray(input, np.float32)
    in_maps = _make_in_maps(input, Wq, bq, Wk, bk, Wv, bv)
    res = run_bass_kernel_spmd(nc, in_maps, core_ids=list(range(NCORES)),
                               trace=_trace)
    read = np.stack([r["out"] for r in res.results], axis=0)
    out = np.concatenate((input, read), axis=2)
    if _trace:
        kernel.last_result = res
    return out
